# revision 1
# baseline (speedup 1.0000x reference)
"""Trainium2 Bass kernel for Attention1D (visual-question attention).

Computation (per batch b):
    X2att = X @ W_ques + b_ques                      # [bs, 1024]
    Y2att = Y[b] @ W_vis                             # [512, 1024]
    att   = relu(Y2att + X2att[b])                   # [512, 1024]
    logits= att @ W_map (+ b_map, dropped: softmax shift-invariant)
    w     = softmax(logits)                          # [512]
    out[b]= w @ Y[b]                                 # [2048]

Strategy: data-parallel over batch across 8 cores (32 batches/core).
All matmuls in bf16 (fp32 PSUM accumulation). Host pre-shards and
pre-lays-out the inputs:
  - Y^T (v-major) bf16 for the main matmul rhs (contraction over v
    needs v on partitions); the same tile also feeds the final
    weighted sum on VectorE,
  - weights replicated, X^T for the X2att preamble.
On-chip per batch: 128 accumulating matmuls build att^T in PSUM,
ScalarE applies bias+relu into bf16 SBUF (X2att bias precomputed on PE
with b_ques folded in via a ones-row rank-1 matmul), 8 matmuls
contract att^T with W_map into logits[1, 512], softmax runs on
partition 0 (exp with hardware sum accumulator, DVE reciprocal; the
max-subtraction is dropped since logits are O(1) and softmax is
shift-invariant), the normalized weights are broadcast to all 128
partitions with a rank-1 matmul, and VectorE computes the weighted
sum over the resident Y^T tile (multiply + free-dim reduction), so Y
is read from HBM exactly once. A post-pass splits multi-wait
instructions into single-wait form (this walrus build allows one sync
wait per instruction).
"""

import numpy as np
import ml_dtypes

BS, N = 256, 512
QD, VD, AD = 2048, 2048, 1024
NCORES = 8
BPC = BS // NCORES  # batches per core
VC, QC, AC, NC_ = VD // 128, QD // 128, AD // 128, N // 128  # chunk counts

BF = ml_dtypes.bfloat16

_cache = {}


def _split_multiwait(nc, mybir):
    """walrus codegen in this toolchain supports a single sync-wait per
    instruction; hoist extra waits onto standalone same-engine
    EventSemaphore waits placed immediately before the instruction."""
    k = 0
    for f in nc.m.functions:
        for blk in f.blocks:
            il = blk.instructions
            new = []
            for inst in il:
                si = inst.sync_info
                waits = list(si.on_wait) if si and si.on_wait else []
                if len(waits) > 1:
                    for w in waits[:-1]:
                        k += 1
                        ev = mybir.InstEventSemaphore(
                            name=f"antsplitw_{k}",
                            engine=inst.engine,
                            ins=[],
                            outs=[],
                            sync_info=mybir.SyncInfo(on_wait=[w], on_update=[]),
                        )
                        nc.register_instruction(ev, overwrite=True)
                        new.append(ev)
                    si.on_wait = [waits[-1]]
                new.append(inst)
            il[:] = new


def _build_nc():
    import concourse.bass as bass
    import concourse.mybir as mybir
    from concourse import tile
    from contextlib import ExitStack

    f32, bf16 = mybir.dt.float32, mybir.dt.bfloat16
    AF = mybir.ActivationFunctionType

    nc = bass.Bass(target_bir_lowering=True)

    yt_d = nc.declare_dram_parameter("yt", [BPC, VC, 128, N], bf16, isOutput=False)
    wvis_d = nc.declare_dram_parameter("wvis", [VC, 128, AD], bf16, isOutput=False)
    wques_d = nc.declare_dram_parameter("wques", [QC, 128, AD], bf16, isOutput=False)
    wmap_d = nc.declare_dram_parameter("wmap", [128, AC], bf16, isOutput=False)
    bques_d = nc.declare_dram_parameter("bques", [128, AC], f32, isOutput=False)
    xt_d = nc.declare_dram_parameter("xt", [QC, 128, BPC], bf16, isOutput=False)
    ones_d = nc.declare_dram_parameter("ones", [1, 128], bf16, isOutput=False)
    bqrow_d = nc.declare_dram_parameter("bqrow", [1, AD], bf16, isOutput=False)
    out_d = nc.declare_dram_parameter("out", [BPC, VD], f32, isOutput=True)

    with tile.TileContext(nc) as tc, ExitStack() as ctx:
        consts = ctx.enter_context(tc.tile_pool(name="consts", bufs=1))
        yt_pool = ctx.enter_context(tc.tile_pool(name="yt", bufs=3))
        att_pool = ctx.enter_context(tc.tile_pool(name="att", bufs=2))
        sm_pool = ctx.enter_context(tc.tile_pool(name="sm", bufs=3))
        ob_pool = ctx.enter_context(tc.tile_pool(name="ob", bufs=2))
        psA = ctx.enter_context(tc.tile_pool(name="psA", bufs=4, space="PSUM"))
        psL = ctx.enter_context(tc.tile_pool(name="psL", bufs=2, space="PSUM"))
        psW = ctx.enter_context(tc.tile_pool(name="psW", bufs=2, space="PSUM"))

        # ---- load constants ----
        wvis_sb = consts.tile([128, VC * AD], bf16, tag="wvis")
        nc.sync.dma_start(
            wvis_sb.rearrange("p (v a) -> p v a", v=VC),
            wvis_d.rearrange("v p a -> p v a"),
        )
        wques_sb = consts.tile([128, QC * AD], bf16, tag="wques")
        nc.sync.dma_start(
            wques_sb.rearrange("p (q a) -> p q a", q=QC),
            wques_d.rearrange("q p a -> p q a"),
        )
        wmap_sb = consts.tile([128, AC], bf16, tag="wmap")
        nc.sync.dma_start(wmap_sb[:], wmap_d[:])
        ones_sb = consts.tile([1, 128], bf16, tag="ones")
        nc.sync.dma_start(ones_sb[:], ones_d[:])
        bqrow_sb = consts.tile([1, AD], bf16, tag="bqrow")
        nc.sync.dma_start(bqrow_sb[:], bqrow_d[:])
        xt_sb = consts.tile([128, QC * BPC], bf16, tag="xt")
        nc.sync.dma_start(
            xt_sb.rearrange("p (q b) -> p q b", q=QC),
            xt_d.rearrange("q p b -> p q b"),
        )

        # ---- preamble: X2att^T [a(8x128 chunks), BPC] fp32, bias folded in ----
        x2att_sb = consts.tile([128, AC * BPC], f32, tag="x2att")
        for a in range(AC):
            ps = psA.tile([128, BPC], f32, tag="main")
            for q in range(QC):
                nc.tensor.matmul(
                    ps[:],
                    wques_sb[:, q * AD + a * 128 : q * AD + (a + 1) * 128],
                    xt_sb[:, q * BPC : (q + 1) * BPC],
                    start=(q == 0),
                    stop=False,
                )
            nc.tensor.matmul(
                ps[:],
                bqrow_sb[0:1, a * 128 : (a + 1) * 128],
                ones_sb[0:1, 0:BPC],
                start=False,
                stop=True,
            )
            nc.vector.tensor_copy(
                x2att_sb[:, a * BPC : (a + 1) * BPC], ps[:]
            )
        # one-time ACT observer of DVE-produced x2att so later relu ACTs
        # carry only the PE wait (walrus ACT codegen allows 1 sync wait)
        x2obs = consts.tile([128, 1], f32, tag="x2obs")
        nc.scalar.copy(x2obs[:], x2att_sb[:, 0:1])

        # ---- main loop over batches ----
        for b in range(BPC):
            yt = yt_pool.tile([128, VC * N], bf16)
            nc.sync.dma_start(
                yt.rearrange("p (v n) -> p v n", v=VC),
                yt_d[b].rearrange("v p n -> p v n"),
            )
            att = att_pool.tile([128, AC * N], bf16)
            for a in range(AC):
                ps = psA.tile([128, N], f32, tag="main")
                for v in range(VC):
                    nc.tensor.matmul(
                        ps[:],
                        wvis_sb[:, v * AD + a * 128 : v * AD + (a + 1) * 128],
                        yt[:, v * N : (v + 1) * N],
                        start=(v == 0),
                        stop=(v == VC - 1),
                    )
                # att^T chunk = relu(psum + x2att[:, b]) -> bf16
                nc.scalar.activation(
                    att[:, a * N : (a + 1) * N],
                    ps[:],
                    AF.Relu,
                    bias=x2att_sb[:, a * BPC + b : a * BPC + b + 1],
                )

            # logits [1, N] = sum_a wmap[a_chunk]^T @ att^T[a_chunk]
            psl = psL.tile([1, N], f32)
            for a in range(AC):
                nc.tensor.matmul(
                    psl[:],
                    wmap_sb[:, a : a + 1],
                    att[:, a * N : (a + 1) * N],
                    start=(a == 0),
                    stop=(a == AC - 1),
                )

            # softmax on partition 0; logits are O(1) so exp needs no
            # max-subtraction (softmax is shift-invariant, fp32 exact enough)
            e_sb = sm_pool.tile([1, N], f32, tag="e")
            ssum = sm_pool.tile([1, 1], f32, tag="ssum")
            nc.scalar.activation(
                e_sb[:], psl[:], AF.Exp, accum_out=ssum[:]
            )
            rcp = sm_pool.tile([1, 1], f32, tag="rcp")
            nc.vector.reciprocal(rcp[:], ssum[:])
            rcp_a = sm_pool.tile([1, 1], f32, tag="rcp_a")
            nc.scalar.copy(rcp_a[:], rcp[:])
            e_w = sm_pool.tile([1, N], bf16, tag="e_w")
            nc.scalar.mul(e_w[:], e_sb[:], rcp_a[:, 0:1])

            # broadcast w = e/s to all 128 partitions via rank-1 matmul
            psw = psW.tile([128, N], f32)
            nc.tensor.matmul(psw[:], ones_sb[:], e_w[:], start=True, stop=True)
            wbc = sm_pool.tile([128, N], bf16, tag="wbc")
            nc.scalar.copy(wbc[:], psw[:])

            # weighted sum on DVE over the resident Y^T tile:
            # out^T[v_chunk*128+p] = sum_n Y^T[vp, n] * w[n]
            ob = ob_pool.tile([128, VC], f32)
            prod = sm_pool.tile([128, VC * N], bf16, tag="prod")
            nc.vector.tensor_tensor(
                prod[:].rearrange("p (c n) -> p c n", c=VC),
                yt[:].rearrange("p (c n) -> p c n", c=VC),
                wbc[:].rearrange("p (o n) -> p o n", o=1).broadcast_to(
                    [128, VC, N]
                ),
                op=mybir.AluOpType.mult,
            )
            nc.vector.reduce_sum(
                ob[:],
                prod[:].rearrange("p (c n) -> p c n", c=VC),
                axis=mybir.AxisListType.X,
            )
            nc.sync.dma_start(
                out_d[b].rearrange("(c p) -> p c", p=128), ob[:]
            )

    _split_multiwait(nc, mybir)
    return nc


def _prep_core_inputs(X, Y, W_vis, W_ques, b_ques, W_map):
    """Build per-core input maps (host-side shard + layout + bf16 cast)."""
    wvis_h = np.ascontiguousarray(W_vis.reshape(VC, 128, AD)).astype(BF)
    wques_h = np.ascontiguousarray(W_ques.reshape(QC, 128, AD)).astype(BF)
    wmap_h = np.ascontiguousarray(W_map.reshape(AC, 128).T).astype(BF)
    bques_h = np.ascontiguousarray(b_ques.reshape(AC, 128).T).astype(np.float32)
    bques_h_row = np.ascontiguousarray(b_ques.reshape(1, AD)).astype(BF)

    in_maps = []
    for c in range(NCORES):
        sl = slice(c * BPC, (c + 1) * BPC)
        Yc = Y[sl]  # [BPC, N, VD]
        yt = np.ascontiguousarray(Yc.transpose(0, 2, 1)).reshape(
            BPC, VC, 128, N
        ).astype(BF)
        xt = np.ascontiguousarray(X[sl].T).reshape(QC, 128, BPC).astype(BF)
        in_maps.append(
            {
                "yt": yt,
                "wvis": wvis_h,
                "wques": wques_h,
                "wmap": wmap_h,
                "bques": bques_h,
                "xt": xt,
                "ones": np.ones((1, 128), dtype=BF),
                "bqrow": bques_h_row,
            }
        )
    return in_maps


def _get_nc():
    if "nc" not in _cache:
        _cache["nc"] = _build_nc()
    return _cache["nc"]


def kernel(X, Y, W_vis, W_ques, b_ques, W_map, b_map, _trace=False):
    from concourse.bass_utils import run_bass_kernel_spmd

    X = np.asarray(X, dtype=np.float32)
    Y = np.asarray(Y, dtype=np.float32)
    in_maps = _prep_core_inputs(
        np.asarray(X), np.asarray(Y), np.asarray(W_vis),
        np.asarray(W_ques), np.asarray(b_ques), np.asarray(W_map)
    )
    nc = _get_nc()
    res = run_bass_kernel_spmd(
        nc, in_maps, core_ids=list(range(NCORES)), trace=_trace
    )
    if _trace:
        _cache["last_result"] = res
    out = np.concatenate([r["out"] for r in res.results], axis=0)
    # b_map shifts logits uniformly -> softmax-invariant; output unaffected.
    return out.astype(np.float32)



# revision 14
# speedup vs baseline: 3.8022x; 3.8022x over previous
"""Trainium2 Bass kernel for Attention1D (visual-question attention).

Computation (per batch b):
    X2att = X @ W_ques + b_ques                      # [bs, 1024]
    Y2att = Y[b] @ W_vis                             # [512, 1024]
    att   = relu(Y2att + X2att[b])                   # [512, 1024]
    logits= att @ W_map (+ b_map, dropped: softmax shift-invariant)
    w     = softmax(logits)                          # [512]
    out[b]= w @ Y[b]                                 # [2048]

Strategy: data-parallel over batch across 8 cores (32 batches/core).
The dominant matmul (Y @ W_vis, 68.7 GFLOP/core) runs in fp8(e4m3)
with MatmulPerfMode.DoubleRow: each PE instruction contracts 256 rows
(two 128-row k-tiles) at half the per-column cycle cost, 4x the bf16
column rate. Y ships twice on parallel DMA queues: fp8 in Y^T layout
(SP queue) for the main matmul's moving operand, bf16 in natural
n-major layout (ACT queue) for the weighted sum.

Per batch on-chip:
  - 64 DoubleRow matmuls build att^T chunks [128a, 512n] in PSUM,
  - DVE tensor_scalar applies (+X2att bias, max 0) PSUM->fp8 SBUF
    (GPSIMD cannot access PSUM on TRN2; Pool only copies/DMAs),
  - 4 DoubleRow matmuls against a 32x-scaled fp8 W_map padded to the
    full 128-wide stationary (zeros in columns 1..127 — the narrow
    m=1 fp8 ldweights AP fails the walrus ISA check) give the logits
    in PSUM row 0; the 1/32 folds into the exp's scale,
  - ACT exp + hw sum accumulator, DVE reciprocal + normalize (bf16 w),
  - 4 rank-1 matmuls transpose w into per-partition columns [128,4],
  - the weighted sum runs on the PE as 64 single-column matmuls
    (stationary = resident bf16 Y[n,v] 128x128 block, moving = the
    w column for that n-chunk, ~1 cycle each) accumulating out^T
    [128,16] over the four n-chunks in one PSUM bank.
X2att is computed on-device in a small fp8 DoubleRow preamble with the
b_ques bias folded in during the DVE PSUM drain. A post-pass splits
multi-wait instructions into single-wait form (walrus codegen allows
one sync wait per instruction).
"""

import numpy as np
import ml_dtypes

BS, N = 256, 512
QD, VD, AD = 2048, 2048, 1024
NCORES = 8
BPC = BS // NCORES  # batches per core
VC, QC, AC, NC_ = VD // 128, QD // 128, AD // 128, N // 128  # 16, 16, 8, 4

BF = ml_dtypes.bfloat16
F8 = ml_dtypes.float8_e4m3
WMAP_SCALE = 32.0

_cache = {}


def _split_multiwait(nc, mybir):
    """walrus codegen in this toolchain supports a single sync-wait per
    instruction; hoist extra waits onto standalone same-engine
    EventSemaphore waits placed immediately before the instruction."""
    k = 0
    for f in nc.m.functions:
        for blk in f.blocks:
            il = blk.instructions
            new = []
            for inst in il:
                si = inst.sync_info
                waits = list(si.on_wait) if si and si.on_wait else []
                if len(waits) > 1:
                    for w in waits[:-1]:
                        k += 1
                        ev = mybir.InstEventSemaphore(
                            name=f"antsplitw_{k}",
                            engine=inst.engine,
                            ins=[],
                            outs=[],
                            sync_info=mybir.SyncInfo(on_wait=[w], on_update=[]),
                        )
                        nc.register_instruction(ev, overwrite=True)
                        new.append(ev)
                    si.on_wait = [waits[-1]]
                new.append(inst)
            il[:] = new


def _build_nc():
    import concourse.bass as bass
    import concourse.mybir as mybir
    from concourse import tile
    from contextlib import ExitStack

    f32, bf16, f8 = mybir.dt.float32, mybir.dt.bfloat16, mybir.dt.float8e4
    AF = mybir.ActivationFunctionType
    AL = mybir.AluOpType
    DR = mybir.MatmulPerfMode.DoubleRow

    nc = bass.Bass(target_bir_lowering=True)

    y8_d = nc.declare_dram_parameter("y8", [BPC, VC, 128, N], f8, isOutput=False)
    ynv_d = nc.declare_dram_parameter("ynv", [BPC, NC_, 128, VD], bf16, isOutput=False)
    wvis_d = nc.declare_dram_parameter("wvis", [VC, 128, AD], f8, isOutput=False)
    wques_d = nc.declare_dram_parameter("wques", [QC, 128, AD], f8, isOutput=False)
    xt_d = nc.declare_dram_parameter("xt", [QC, 128, BPC], f8, isOutput=False)
    bques_d = nc.declare_dram_parameter("bques", [128, AC], f32, isOutput=False)
    # wide stationary for the logits matmul: W_map in column m=0, zeros in
    # m=1..127 (narrow m=1 fp8 DR ldweights fail the walrus ISA check)
    wmap_d = nc.declare_dram_parameter("wmap", [128, AC, 128], f8, isOutput=False)
    ones_d = nc.declare_dram_parameter("ones", [1, 128], bf16, isOutput=False)
    out_d = nc.declare_dram_parameter("out", [BPC, VD], f32, isOutput=True)

    with tile.TileContext(nc) as tc, ExitStack() as ctx:
        consts = ctx.enter_context(tc.tile_pool(name="consts", bufs=1))
        y8_pool = ctx.enter_context(tc.tile_pool(name="y8", bufs=3))
        ynv_pool = ctx.enter_context(tc.tile_pool(name="ynv", bufs=3))
        att_pool = ctx.enter_context(tc.tile_pool(name="att", bufs=2))
        sm_pool = ctx.enter_context(tc.tile_pool(name="sm", bufs=2))
        ob_pool = ctx.enter_context(tc.tile_pool(name="ob", bufs=2))
        psA = ctx.enter_context(tc.tile_pool(name="psA", bufs=3, space="PSUM"))
        psL = ctx.enter_context(tc.tile_pool(name="psL", bufs=1, space="PSUM"))
        psT = ctx.enter_context(tc.tile_pool(name="psT", bufs=2, space="PSUM"))
        psO = ctx.enter_context(tc.tile_pool(name="psO", bufs=2, space="PSUM"))

        # ---- load constants ----
        wvis_sb = consts.tile([128, VC * AD], f8, tag="wvis")
        nc.sync.dma_start(
            wvis_sb.rearrange("p (v a) -> p v a", v=VC),
            wvis_d.rearrange("v p a -> p v a"),
        )
        wques_sb = consts.tile([128, QC * AD], f8, tag="wques")
        nc.sync.dma_start(
            wques_sb.rearrange("p (q a) -> p q a", q=QC),
            wques_d.rearrange("q p a -> p q a"),
        )
        xt_sb = consts.tile([128, QC * BPC], f8, tag="xt")
        nc.sync.dma_start(
            xt_sb.rearrange("p (q b) -> p q b", q=QC),
            xt_d.rearrange("q p b -> p q b"),
        )
        bques_sb = consts.tile([128, AC], f32, tag="bques")
        nc.sync.dma_start(bques_sb[:], bques_d[:])
        wmap_sb = consts.tile([128, AC * 128], f8, tag="wmap")
        nc.sync.dma_start(
            wmap_sb.rearrange("p (c m) -> p c m", c=AC), wmap_d[:]
        )
        ones_sb = consts.tile([1, 128], bf16, tag="ones")
        nc.sync.dma_start(ones_sb[:], ones_d[:])

        wv_r = wvis_sb.rearrange("p (v a) -> p v a", v=VC)
        wq_r = wques_sb.rearrange("p (q a) -> p q a", q=QC)
        xt_r = xt_sb.rearrange("p (q b) -> p q b", q=QC)
        wm_r = wmap_sb.rearrange("p (c m) -> p c m", c=AC)

        # ---- preamble: X2att^T [a(8x128 chunks), BPC] fp32, bias on drain ----
        x2att_sb = consts.tile([128, AC * BPC], f32, tag="x2att")
        for a in range(AC):
            ps = psA.tile([128, BPC], f32, tag="A")
            for q in range(QC // 2):
                nc.tensor.matmul(
                    ps[:],
                    wq_r[:, 2 * q : 2 * q + 2, a * 128 : (a + 1) * 128],
                    xt_r[:, 2 * q : 2 * q + 2, :],
                    start=(q == 0),
                    stop=(q == QC // 2 - 1),
                    perf_mode=DR,
                )
            nc.vector.tensor_scalar(
                x2att_sb[:, a * BPC : (a + 1) * BPC],
                ps[:],
                bques_sb[:, a : a + 1],
                0.0,
                op0=AL.add,
                op1=AL.add,
            )

        # ---- main loop over batches ----
        for b in range(BPC):
            y8 = y8_pool.tile([128, VC * N], f8)
            nc.sync.dma_start(
                y8.rearrange("p (v n) -> p v n", v=VC),
                y8_d[b].rearrange("v p n -> p v n"),
            )
            y8_r = y8.rearrange("p (v n) -> p v n", v=VC)
            ynv = ynv_pool.tile([128, NC_ * VD], bf16)
            nc.scalar.dma_start(
                ynv.rearrange("p (c v) -> p c v", c=NC_),
                ynv_d[b].rearrange("c p v -> p c v"),
            )

            att = att_pool.tile([128, AC * N], f8)
            att_r = att.rearrange("p (c n) -> p c n", c=AC)
            for a in range(AC):
                ps = psA.tile([128, N], f32, tag="A")
                for v in range(VC // 2):
                    nc.tensor.matmul(
                        ps[:],
                        wv_r[:, 2 * v : 2 * v + 2, a * 128 : (a + 1) * 128],
                        y8_r[:, 2 * v : 2 * v + 2, :],
                        start=(v == 0),
                        stop=(v == VC // 2 - 1),
                        perf_mode=DR,
                    )
                # att chunk = relu(psum + x2att[:, b]) -> fp8, on DVE
                nc.vector.tensor_scalar(
                    att[:, a * N : (a + 1) * N],
                    ps[:],
                    x2att_sb[:, a * BPC + b : a * BPC + b + 1],
                    0.0,
                    op0=AL.add,
                    op1=AL.max,
                )

            # logits = sum_a (32*wmap)^T @ att^T in psl row 0 (rows 1..127
            # are zero-weight junk); /32 applied in the exp scale
            psl = psL.tile([128, N], f32)
            for j in range(AC // 2):
                nc.tensor.matmul(
                    psl[:],
                    wm_r[:, 2 * j : 2 * j + 2, :],
                    att_r[:, 2 * j : 2 * j + 2, :],
                    start=(j == 0),
                    stop=(j == AC // 2 - 1),
                    perf_mode=DR,
                )

            # softmax; logits are O(1) so exp needs no max-subtraction
            e_sb = sm_pool.tile([1, N], f32, tag="e")
            ssum = sm_pool.tile([1, 1], f32, tag="ssum")
            nc.scalar.activation(
                e_sb[:], psl[0:1, :], AF.Exp, scale=1.0 / WMAP_SCALE,
                accum_out=ssum[:],
            )
            rcp = sm_pool.tile([1, 1], f32, tag="rcp")
            nc.vector.reciprocal(rcp[:], ssum[:])
            e_w = sm_pool.tile([1, N], bf16, tag="e_w")
            nc.vector.tensor_scalar(
                e_w[:], e_sb[:], rcp[:, 0:1], None, op0=AL.mult
            )

            # transpose w into per-partition columns [128, NC_] via rank-1
            # outer products, drain to bf16
            pst = psT.tile([128, NC_], f32)
            for n in range(NC_):
                nc.tensor.matmul(
                    pst[:, n : n + 1],
                    e_w[0:1, n * 128 : (n + 1) * 128],
                    ones_sb[0:1, 0:1],
                    start=True,
                    stop=True,
                )
            wcols = sm_pool.tile([128, NC_], bf16, tag="wcols")
            nc.vector.tensor_copy(wcols[:], pst[:])

            # weighted sum on PE: out^T[v*128+p] accumulated over n-chunks,
            # stationary = Y[n,v] 128x128 block, moving = w column (1 col)
            pso = psO.tile([128, VC], f32)
            for v in range(VC):
                for n in range(NC_):
                    nc.tensor.matmul(
                        pso[:, v : v + 1],
                        ynv[:, n * VD + v * 128 : n * VD + (v + 1) * 128],
                        wcols[:, n : n + 1],
                        start=(n == 0),
                        stop=(n == NC_ - 1),
                    )
            ob = ob_pool.tile([128, VC], f32)
            nc.vector.tensor_copy(ob[:], pso[:])
            nc.gpsimd.dma_start(
                out_d[b].rearrange("(c p) -> p c", p=128), ob[:]
            )

    _split_multiwait(nc, mybir)
    return nc


def _prep_core_inputs(X, Y, W_vis, W_ques, b_ques, W_map):
    """Build per-core input maps (host-side shard + layout + casts)."""
    wvis_h = np.ascontiguousarray(W_vis.reshape(VC, 128, AD)).astype(F8)
    wques_h = np.ascontiguousarray(W_ques.reshape(QC, 128, AD)).astype(F8)
    bques_h = np.ascontiguousarray(b_ques.reshape(AC, 128).T).astype(np.float32)
    wmap_h = np.zeros((128, AC, 128), dtype=F8)
    wmap_h[:, :, 0] = (W_map * WMAP_SCALE).reshape(AC, 128).T.astype(F8)
    ones_h = np.ones((1, 128), dtype=BF)

    in_maps = []
    for c in range(NCORES):
        sl = slice(c * BPC, (c + 1) * BPC)
        Yc = Y[sl]  # [BPC, N, VD]
        y8 = np.ascontiguousarray(Yc.transpose(0, 2, 1)).reshape(
            BPC, VC, 128, N
        ).astype(F8)
        ynv = Yc.reshape(BPC, NC_, 128, VD).astype(BF)
        xt = np.ascontiguousarray(X[sl].T).reshape(QC, 128, BPC).astype(F8)
        in_maps.append(
            {
                "y8": y8,
                "ynv": ynv,
                "wvis": wvis_h,
                "wques": wques_h,
                "xt": xt,
                "bques": bques_h,
                "wmap": wmap_h,
                "ones": ones_h,
            }
        )
    return in_maps


def _get_nc():
    if "nc" not in _cache:
        _cache["nc"] = _build_nc()
    return _cache["nc"]


def kernel(X, Y, W_vis, W_ques, b_ques, W_map, b_map, _trace=False):
    from concourse.bass_utils import run_bass_kernel_spmd

    X = np.asarray(X, dtype=np.float32)
    Y = np.asarray(Y, dtype=np.float32)
    in_maps = _prep_core_inputs(
        np.asarray(X), np.asarray(Y), np.asarray(W_vis),
        np.asarray(W_ques), np.asarray(b_ques), np.asarray(W_map)
    )
    nc = _get_nc()
    res = run_bass_kernel_spmd(
        nc, in_maps, core_ids=list(range(NCORES)), trace=_trace
    )
    if _trace:
        _cache["last_result"] = res
    out = np.concatenate([r["out"] for r in res.results], axis=0)
    # b_map shifts logits uniformly -> softmax-invariant; output unaffected.
    return out.astype(np.float32)
